# revision 3
# baseline (speedup 1.0000x reference)
"""Trainium2 Bass kernel for nn_ActualChunkedAttention (8 NeuronCores).

Chunked (flash-style) causal attention, B=2 T=2048 D=1024 H=16 Dh=64.
The reference's streaming online-softmax over kv chunks is mathematically
exact dense causal attention, so we compute it directly (scores are tiny:
|s/sqrt(d)| < ~3, so no max-subtraction is needed for fp32/bf16 exp).

Sharding (no cross-device comms): core c -> batch b=c//4, head-group
hg=c%4 (heads 4*hg..4*hg+3). Each core computes its Q/K/V projections,
attention, and a PARTIAL Wo projection over its 256 d_model-in dims;
the host sums the 4 partials per batch (pure unsharding of the
row-parallel Wo).

Per-core kernel (all matmul operands bf16, fp32 PSUM):
  - x^T, W slices pre-transposed on host; Q^T/K^T computed in
    d-on-partitions layout ([128, T] per head-pair), V in natural [kv, d]
    layout with an appended ones column so each PV matmul also
    accumulates the softmax denominator l as row 64.
  - S^T[kv, q] per 128-kv-chunk via row-paired K=64 matmuls (two heads
    concurrently in the PE array); exp on ScalarE with the 1/8 scale
    folded in; causal masking via an additive -480 triangular tile on
    the diagonal blocks (pre-exp, off the exp->PV critical path) plus
    column restriction of S/exp/PV to valid columns.
  - attention emitted as one flat software-pipelined (qi, pair, kb)
    stream: S(i+1) always precedes PV(i) in PE program order so the
    in-order PE never head-of-line blocks on exp.
  - 1/l via a pack-to-[128,8] DVE reciprocal (DVE reciprocal costs ~9
    cycles per FREE element, so make the free dim tiny), broadcast back
    through DRAM (partition-broadcast APs need a DRAM source).
  - Wo with the yT subtile stationary (half the LDWEIGHTS), producing
    the partial output in natural [T, 1024] orientation.
"""

import ml_dtypes
import numpy as np

import concourse.bass as bass
import concourse.mybir as mybir
import concourse.tile as tile
from concourse import bacc
from concourse.bass_utils import run_bass_kernel_spmd

BF = mybir.dt.bfloat16
F32 = mybir.dt.float32
AF = mybir.ActivationFunctionType
BF16 = ml_dtypes.bfloat16

B, T, DM, H, DH = 2, 2048, 1024, 16, 64
N_CORES = 8

_cache = {}


def _build(T=2048):
    DM = 1024
    KCH = DM // 128
    NQ = T // 512
    NKV = T // 128

    nc = bacc.Bacc("TRN2", target_bir_lowering=False, debug=False, num_devices=8)
    xT_ext = nc.declare_dram_parameter("xT", [DM, T], BF, isOutput=False)
    wqT_ext = nc.declare_dram_parameter("wqT", [DM, 256], BF, isOutput=False)
    wkT_ext = nc.declare_dram_parameter("wkT", [DM, 256], BF, isOutput=False)
    wvT_ext = nc.declare_dram_parameter("wvT", [DM, 256], BF, isOutput=False)
    woT_ext = nc.declare_dram_parameter("woT", [256, DM], BF, isOutput=False)
    trineg_ext = nc.declare_dram_parameter("trineg", [128, 128], BF, isOutput=False)
    out_ext = nc.declare_dram_parameter("out", [T, DM], F32, isOutput=True)

    with tile.TileContext(nc) as tc:
        with (
            tc.tile_pool(name="persist", bufs=1) as persist,
            tc.tile_pool(name="ptp", bufs=8) as ptp,
            tc.tile_pool(name="ostp", bufs=4) as ostp,
            tc.tile_pool(name="scp", bufs=4) as scp,
            tc.tile_pool(name="dramp", bufs=1, space="DRAM") as dramp,
        ):
            # preload ACT exp table early
            dummy = persist.tile([1, 8], F32, name="dummy")
            nc.vector.memset(dummy[:], 0.0)
            nc.scalar.activation(out=dummy[:], in_=dummy[:], func=AF.Exp)

            # ---- input loads: wq/wk first, xT streamed per k-chunk
            wq_sb = persist.tile([128, KCH, 256], BF, name="wq_sb")
            wk_sb = persist.tile([128, KCH, 256], BF, name="wk_sb")
            nc.sync.dma_start(
                out=wq_sb[:], in_=wqT_ext[:, :].rearrange("(k p) n -> p k n", p=128)
            )
            nc.sync.dma_start(
                out=wk_sb[:], in_=wkT_ext[:, :].rearrange("(k p) n -> p k n", p=128)
            )
            xT_sb = persist.tile([128, KCH, T], BF, name="xT_sb")
            xT_r = xT_ext[:, :].rearrange("(k p) n -> p k n", p=128)
            for k in range(KCH):
                nc.sync.dma_start(out=xT_sb[:, k, :], in_=xT_r[:, k, :])
            trineg_sb = persist.tile([128, 128], BF, name="trineg_sb")
            nc.sync.dma_start(out=trineg_sb[:], in_=trineg_ext[:, :])
            wv_sb = persist.tile([128, KCH, 256], BF, name="wv_sb")
            nc.sync.dma_start(
                out=wv_sb[:], in_=wvT_ext[:, :].rearrange("(k p) n -> p k n", p=128)
            )
            wo_sb = persist.tile([128, 2, DM], BF, name="wo_sb")
            nc.sync.dma_start(
                out=wo_sb[:], in_=woT_ext[:, :].rearrange("(k p) n -> p k n", p=128)
            )

            QT = [persist.tile([128, T], BF, name=f"QT{p}") for p in range(2)]
            KT = [persist.tile([128, T], BF, name=f"KT{p}") for p in range(2)]
            V_sb = persist.tile([128, NKV, 4, 65], BF, name="V_sb")
            yT = [persist.tile([128, T], BF, name=f"yT{p}") for p in range(2)]
            lrow_t = persist.tile([65, 2, 512], F32, name="lrow_t")

            for kc in range(NKV):
                nc.vector.memset(V_sb[:, kc, :, 64:65], 1.0)

            with tc.tile_pool(name="psA", bufs=1, space="PSUM") as psA:
                # PE warmup: junk matmuls while xT streams in, so the HAM
                # clock gate is released before real work
                warm = psA.tile([128, 512], F32, tag="acc8", bufs=8, name="warm")
                for w in range(16):
                    nc.tensor.matmul(
                        warm[:, 0:256],
                        lhsT=wq_sb[:, 0, 0:128],
                        rhs=wq_sb[:, w % 8, :],
                        start=True,
                        stop=True,
                    )
                # pair-0 Q/K: k-outer so matmuls start as xT chunks land
                accQ = [
                    psA.tile([128, 512], F32, tag="acc8", bufs=8, name=f"accQ{qi}")
                    for qi in range(NQ)
                ]
                accK = [
                    psA.tile([128, 512], F32, tag="acc8", bufs=8, name=f"accK{qi}")
                    for qi in range(NQ)
                ]
                for k in range(KCH):
                    for qi in range(NQ):
                        nc.tensor.matmul(
                            accQ[qi][:],
                            lhsT=wq_sb[:, k, 0:128],
                            rhs=xT_sb[:, k, 512 * qi : 512 * (qi + 1)],
                            start=(k == 0),
                            stop=(k == KCH - 1),
                        )
                        nc.tensor.matmul(
                            accK[qi][:],
                            lhsT=wk_sb[:, k, 0:128],
                            rhs=xT_sb[:, k, 512 * qi : 512 * (qi + 1)],
                            start=(k == 0),
                            stop=(k == KCH - 1),
                        )
                for qi in range(NQ):
                    nc.vector.tensor_copy(
                        out=QT[0][:, 512 * qi : 512 * (qi + 1)], in_=accQ[qi][:]
                    )
                    nc.vector.tensor_copy(
                        out=KT[0][:, 512 * qi : 512 * (qi + 1)], in_=accK[qi][:]
                    )
                # V in natural [kv, d] orientation, strided into 65-wide slots
                for kc in range(NKV):
                    ps = psA.tile([128, 256], F32, tag="acc8", bufs=8, name="ps_v")
                    for k in range(KCH):
                        nc.tensor.matmul(
                            ps[:],
                            lhsT=xT_sb[:, k, 128 * kc : 128 * (kc + 1)],
                            rhs=wv_sb[:, k, :],
                            start=(k == 0),
                            stop=(k == KCH - 1),
                        )
                    nc.vector.tensor_copy(
                        out=V_sb[:, kc, :, 0:64],
                        in_=ps[:].rearrange("p (h d) -> p h d", h=4),
                    )
                # pair-1 Q/K
                for qi in range(NQ):
                    for w_sb, dest in ((wq_sb, QT[1]), (wk_sb, KT[1])):
                        ps = psA.tile(
                            [128, 512], F32, tag="acc8", bufs=8, name="ps_p1"
                        )
                        for k in range(KCH):
                            nc.tensor.matmul(
                                ps[:],
                                lhsT=w_sb[:, k, 128:256],
                                rhs=xT_sb[:, k, 512 * qi : 512 * (qi + 1)],
                                start=(k == 0),
                                stop=(k == KCH - 1),
                            )
                        nc.vector.tensor_copy(
                            out=dest[:, 512 * qi : 512 * (qi + 1)], in_=ps[:]
                        )

            # ---- attention: flat software-pipelined (qi, pair, kb) stream
            steps = []
            for qi in range(NQ):
                for p in range(2):
                    for kb in range((qi + 1) * 4):
                        steps.append((qi, p, kb))
            n = len(steps)

            with tc.tile_pool(name="psB", bufs=1, space="PSUM") as psB:

                def emit_wo(qi):
                    for sub in range(4):  # 128-row q subtiles
                        qs = slice(
                            512 * qi + 128 * sub, 512 * qi + 128 * (sub + 1)
                        )
                        ps = [
                            psB.tile(
                                [128, 512], F32, tag="o_ps", bufs=2,
                                name=f"ps_wo{half}",
                            )
                            for half in range(2)
                        ]
                        for ic in range(2):
                            for half in range(2):
                                nc.tensor.matmul(
                                    ps[half][:],
                                    lhsT=yT[ic][:, qs],
                                    rhs=wo_sb[:, ic, 512 * half : 512 * (half + 1)],
                                    start=(ic == 0),
                                    stop=(ic == 1),
                                )
                        for half in range(2):
                            ost = ostp.tile([128, 512], F32, name="ost")
                            nc.vector.tensor_copy(out=ost[:], in_=ps[half][:])
                            nc.sync.dma_start(
                                out=out_ext[qs, 512 * half : 512 * (half + 1)],
                                in_=ost[:],
                            )

                def alloc_s():
                    return psB.tile(
                        [128, 1024], F32, tag="s_pair", bufs=3, name="s_pair"
                    )

                def emit_s(S, step):
                    qi, p, kb = step
                    off = max(0, 128 * kb - 512 * qi)
                    for h in range(2):
                        sl = slice(64 * h, 64 * (h + 1))
                        nc.tensor.matmul(
                            S[:, 512 * h + off : 512 * (h + 1)],
                            lhsT=KT[p][sl, 128 * kb : 128 * (kb + 1)],
                            rhs=QT[p][sl, 512 * qi + off : 512 * (qi + 1)],
                            start=True,
                            stop=True,
                        )

                O_ps = None
                S_tiles = {}
                S_tiles[0] = alloc_s()
                emit_s(S_tiles[0], steps[0])
                S_tiles[1] = alloc_s()
                emit_s(S_tiles[1], steps[1])
                for i, (qi, p, kb) in enumerate(steps):
                    qsl = slice(512 * qi, 512 * (qi + 1))
                    nkv = (qi + 1) * 4
                    off = max(0, 128 * kb - 512 * qi)
                    if p == 1 and kb == 0 and qi > 0:
                        emit_wo(qi - 1)
                    if i + 2 < n:
                        S_tiles[i + 2] = alloc_s()
                        emit_s(S_tiles[i + 2], steps[i + 2])
                    S = S_tiles.pop(i)
                    if kb >= qi * 4:  # diagonal chunk: additive causal mask
                        for h in range(2):
                            nc.vector.tensor_add(
                                S[:, 512 * h + off : 512 * h + off + 128],
                                S[:, 512 * h + off : 512 * h + off + 128],
                                trineg_sb[:],
                            )
                    PT = ptp.tile([128, 1024], BF, tag="pt", name="pt")
                    if off > 0:
                        nc.scalar.activation(
                            out=PT[:, :]
                            .rearrange("x (h q) -> x h q", h=2)[:, :, off:512],
                            in_=S[:, :]
                            .rearrange("x (h q) -> x h q", h=2)[:, :, off:512],
                            func=AF.Exp,
                            scale=0.125,
                        )
                    else:
                        nc.scalar.activation(
                            out=PT[:], in_=S[:], func=AF.Exp, scale=0.125
                        )
                    if kb == 0:
                        O_ps = [
                            psB.tile(
                                [65, 512], F32, tag="o_ps", bufs=2, name=f"o_ps{h}"
                            )
                            for h in range(2)
                        ]
                    for h in range(2):
                        nc.tensor.matmul(
                            O_ps[h][:, off:512],
                            lhsT=V_sb[:, kb, 2 * p + h, :],
                            rhs=PT[:, 512 * h + off : 512 * (h + 1)],
                            start=(kb == 0),
                            stop=(kb == nkv - 1),
                        )
                    if kb == nkv - 1:
                        OU = [
                            scp.tile(
                                [64, 512], BF, name=f"OU{h}", tag=f"OU{h}", bufs=4
                            )
                            for h in range(2)
                        ]
                        for h in range(2):
                            nc.vector.tensor_copy(
                                out=lrow_t[64:65, h, :], in_=O_ps[h][64:65, :]
                            )
                            nc.vector.tensor_copy(
                                out=OU[h][:], in_=O_ps[h][0:64, :]
                            )
                        lrec_dram = dramp.tile(
                            [2, 512], F32, name="lrec_dram", tag="lrec_dram", bufs=4
                        )
                        if i == n - 1:
                            # tail: no exps follow -> ACT table switch is
                            # free; 1/l = exp(-ln(l)) skips two DMA hops on
                            # the exposed critical path
                            nc.scalar.activation(
                                out=lrow_t[64:65, :, :],
                                in_=lrow_t[64:65, :, :],
                                func=AF.Ln,
                            )
                            nc.scalar.activation(
                                out=lrow_t[64:65, :, :],
                                in_=lrow_t[64:65, :, :],
                                func=AF.Exp,
                                scale=-1.0,
                            )
                            nc.sync.dma_start(
                                out=lrec_dram[:, :], in_=lrow_t[64:65, :, :]
                            )
                        else:
                            # 1/l: pack the 1024 l values to [128,8] via DRAM
                            # so the DVE reciprocal is partition-parallel
                            l_dram = dramp.tile(
                                [2, 512], F32, name="l_dram", tag="l_dram", bufs=4
                            )
                            nc.sync.dma_start(
                                out=l_dram[:, :], in_=lrow_t[64:65, :, :]
                            )
                            lpack = scp.tile(
                                [128, 8], F32, name="lpack", tag="lpack", bufs=4
                            )
                            nc.sync.dma_start(
                                out=lpack[:],
                                in_=l_dram[:, :].rearrange(
                                    "h (a m) -> (h a) m", m=8
                                ),
                            )
                            lrpack = scp.tile(
                                [128, 8], F32, name="lrpack", tag="lrpack", bufs=4
                            )
                            nc.vector.reciprocal(out=lrpack[:], in_=lpack[:])
                            nc.sync.dma_start(
                                out=lrec_dram[:, :].rearrange(
                                    "h (a m) -> (h a) m", m=8
                                ),
                                in_=lrpack[:],
                            )
                        for h in range(2):
                            src = lrec_dram[h : h + 1, :]
                            bc = bass.AP(
                                tensor=src.tensor,
                                offset=src.offset,
                                ap=[[0, 64], [1, 512]],
                            )
                            lrec = scp.tile(
                                [64, 512], BF, name=f"lrec{h}", tag=f"lrec{h}",
                                bufs=3,
                            )
                            nc.gpsimd.dma_start(out=lrec[:], in_=bc)
                            nc.vector.tensor_mul(
                                yT[p][64 * h : 64 * (h + 1), qsl],
                                OU[h][:],
                                lrec[:],
                            )
                emit_wo(NQ - 1)
    nc.finalize()
    return nc


def _make_trineg():
    # additive pre-exp causal mask for diagonal 128x128 blocks:
    # 0 where valid (c >= p), -480 where masked (exp(0.125*(s-480)) ~= 0)
    p = np.arange(128)[:, None]
    c = np.arange(128)[None, :]
    return np.where(c >= p, 0.0, -480.0).astype(np.float32).astype(BF16)


def kernel(x, Wq, Wk, Wv, Wo):
    x = np.asarray(x, dtype=np.float32)
    Wq = np.asarray(Wq, dtype=np.float32)
    Wk = np.asarray(Wk, dtype=np.float32)
    Wv = np.asarray(Wv, dtype=np.float32)
    Wo = np.asarray(Wo, dtype=np.float32)

    if "nc" not in _cache:
        _cache["nc"] = _build(T)
    nc = _cache["nc"]

    trineg = _make_trineg()
    WqT = np.ascontiguousarray(Wq.T)
    WkT = np.ascontiguousarray(Wk.T)
    WvT = np.ascontiguousarray(Wv.T)
    WoT = np.ascontiguousarray(Wo.T)
    in_maps = []
    for c in range(N_CORES):
        b, hg = c // 4, c % 4
        sl = slice(hg * 256, (hg + 1) * 256)
        in_maps.append(
            {
                "xT": np.ascontiguousarray(x[b].T).astype(BF16),
                "wqT": np.ascontiguousarray(WqT[:, sl]).astype(BF16),
                "wkT": np.ascontiguousarray(WkT[:, sl]).astype(BF16),
                "wvT": np.ascontiguousarray(WvT[:, sl]).astype(BF16),
                "woT": np.ascontiguousarray(WoT[sl, :]).astype(BF16),
                "trineg": trineg,
            }
        )

    res = run_bass_kernel_spmd(nc, in_maps, core_ids=list(range(N_CORES)))

    # unshard: sum the 4 row-parallel Wo partials per batch
    out = np.zeros((B, T, DM), dtype=np.float32)
    for c, r in enumerate(res.results):
        out[c // 4] += r["out"]
    return out
